# revision 6
# baseline (speedup 1.0000x reference)
"""Causal self-attention (B=4, T=2048, C=1024, H=16) on 8 TRN2 NeuronCores.

Sharding: core = (batch b = core//2) x (head-group g = core%2, 8 heads each).
Megatron-style: c_attn column-parallel (each core computes Q/K/V for its 8
heads only), attention local, c_proj row-parallel (each core multiplies its
512 attention-output channels into a full (T, C) partial; host sums the two
partials per batch).

On-chip formulation (everything transposed, channels on partitions):
  qkvT = W^T x^T     : Q^T/K^T as [d, t] tiles (head pairs packed 2x64=128),
                       V as [t, d] tiles with an appended ones column.
  S^T  = K Q^T       : scores transposed [kpos, q]; K=64 contraction, the two
                       heads of a pair run concurrently in PE row-groups 0/64.
  P^T  = exp(S^T/8)  : ScalarE, causal handled by multiplicative masks on the
                       4 diagonal-band tiles per q-chunk (fully-masked tiles
                       are never computed).
  O^T  = Vaug^T P^T  : PSUM-accumulated over kpos; row 64 = softmax
                       denominator (from the ones column of Vaug).
  norm               : reciprocal of row 64, broadcast to 64 partitions via a
                       K=1 matmul, multiply; odd heads shifted to partitions
                       64..127 via SBUF->SBUF DMA.
  outT = Wp^T O^T    : row-parallel projection, written transposed.

All matmuls run as float32r (fp22 mantissa, full PE rate at moving dim 512).
"""

import numpy as np

import concourse.bass as bass
import concourse.mybir as mybir
import concourse.tile as tile
from concourse import bacc
from concourse.bass_utils import run_bass_kernel_spmd

B, T, C, H = 4, 2048, 1024, 16
D = C // H            # 64 head dim
NCORES = 8
NH = H // 2           # 8 heads per core
NP = NH // 2          # 4 head pairs per core
P = 128
QC = 512              # q/t chunk
TCH = T // QC         # 4 chunks
KTILES = T // P       # 16 kpos tiles
CO = C // P           # 8 c-chunks of the model dim
VO = (NH * D) // P    # 4 chunks of the per-core attention channels
F32 = mybir.dt.float32
F32R = mybir.dt.float32r
EXP = mybir.ActivationFunctionType.Exp
SCALE = 1.0 / 8.0     # 1/sqrt(D)

_BUILT = {}


def _r(ap):
    return ap


def _build_bass():
    nc = bacc.Bacc("TRN2", target_bir_lowering=False, debug=False)
    xt_d = nc.dram_tensor("xt", [C, T], F32, kind="ExternalInput")
    wqk_d = nc.dram_tensor("wqk", [C, 2 * NH * D], F32, kind="ExternalInput")
    wv_d = nc.dram_tensor("wv", [C, NH * D], F32, kind="ExternalInput")
    wproj_d = nc.dram_tensor("wproj", [NH * D, C], F32, kind="ExternalInput")
    outT_d = nc.dram_tensor("outT", [C, T], F32, kind="ExternalOutput")

    with tile.TileContext(nc) as tc:
        with tc.tile_pool(name="persist", bufs=1) as persist:
            qt = persist.tile([P, NP, T], F32R)              # Q^T, pair-packed
            kt = persist.tile([P, NP, T], F32R)              # K^T, pair-packed
            v4 = persist.tile([P, KTILES, NH, D + 1], F32R)  # V + ones col
            ones_sb = persist.tile([P, D], F32R)
            one = nc.const_aps.tensor
            nc.vector.tensor_copy(out=ones_sb, in_=one(1.0, (P, D)))
            nc.vector.tensor_copy(out=v4[:, :, :, D],
                                  in_=one(1.0, (P, KTILES, NH)))

            # ---------------- phase 1: QKV projections ----------------
            with tc.tile_pool(name="w1", bufs=1) as w1, \
                 tc.tile_pool(name="xtp", bufs=2) as xtp, \
                 tc.tile_pool(name="ps1", bufs=3, space="PSUM") as ps1:
                wqk_sb = w1.tile([P, CO, 2 * NH * D], F32R)
                nc.gpsimd.dma_start(
                    out=wqk_sb,
                    in_=wqk_d.ap().rearrange("(co ci) n -> ci co n", ci=P))
                wv_sb = w1.tile([P, CO, NH * D], F32R)
                nc.gpsimd.dma_start(
                    out=wv_sb,
                    in_=wv_d.ap().rearrange("(co ci) n -> ci co n", ci=P))

                for tch in range(TCH):
                    tsl = slice(tch * QC, (tch + 1) * QC)
                    xts = xtp.tile([P, CO, QC], F32R, tag="xts")
                    nc.gpsimd.dma_start(
                        out=xts,
                        in_=xt_d.ap()[:, tsl].rearrange(
                            "(co ci) t -> ci co t", ci=P))
                    # Q^T / K^T col-tiles: 0..3 = Q pairs, 4..7 = K pairs
                    for ct in range(8):
                        acc = ps1.tile([P, QC], F32, tag="acc")
                        for c in range(CO):
                            nc.tensor.matmul(
                                acc,
                                lhsT=_r(wqk_sb[:, c, ct * P:(ct + 1) * P]),
                                rhs=_r(xts[:, c, :]),
                                start=(c == 0), stop=(c == CO - 1))
                        dst = qt if ct < NP else kt
                        nc.vector.tensor_copy(
                            out=dst[:, ct % NP, tsl], in_=acc)
                    # V t-tiles
                    for tt in range(QC // P):
                        acc = ps1.tile([P, NH * D], F32, tag="acc")
                        for c in range(CO):
                            nc.tensor.matmul(
                                acc,
                                lhsT=_r(xts[:, c, tt * P:(tt + 1) * P]),
                                rhs=_r(wv_sb[:, c, :]),
                                start=(c == 0), stop=(c == CO - 1))
                        nc.vector.tensor_copy(
                            out=v4[:, tch * (QC // P) + tt, :, 0:D],
                            in_=acc.rearrange("p (h d) -> p h d", h=NH))

            # ---------------- phases 2+3: attention + projection -------
            with tc.tile_pool(name="att", bufs=1) as att, \
                 tc.tile_pool(name="ptp", bufs=3) as ptp, \
                 tc.tile_pool(name="nrm", bufs=2) as nrm, \
                 tc.tile_pool(name="odd", bufs=2) as oddp, \
                 tc.tile_pool(name="ostg", bufs=3) as ostg, \
                 tc.tile_pool(name="ps_sc", bufs=1, space="PSUM") as ps_sc, \
                 tc.tile_pool(name="ps_ot", bufs=4, space="PSUM") as ps_ot, \
                 tc.tile_pool(name="ps_ms", bufs=2, space="PSUM") as ps_ms:
                ot_all = att.tile([P, VO, T], F32R)   # normalized O^T
                masks = att.tile([P, 4, QC], F32)    # diagonal-band masks
                wproj_sb = att.tile([P, VO, C], F32R)
                nc.gpsimd.dma_start(
                    out=wproj_sb,
                    in_=wproj_d.ap().rearrange("(co ci) n -> ci co n", ci=P))
                for di in range(4):
                    m = masks[:, di, :]
                    nc.vector.memset(m, 1.0)
                    # keep where qf - kp - 128*di >= 0
                    nc.gpsimd.affine_select(
                        out=m, in_=m,
                        compare_op=mybir.AluOpType.is_ge,
                        fill=0.0, base=-P * di,
                        pattern=[[1, QC]], channel_multiplier=-1)

                for j in range(TCH):          # q-chunk
                    jsl = slice(j * QC, (j + 1) * QC)
                    for p in range(NP):       # head pair
                        span = 4 * (j + 1)    # causal kpos tiles
                        oth = [ps_ot.tile([D + 1, QC], F32, tag="ot",
                                          name=f"oth{h2}")
                               for h2 in range(2)]
                        for g in range(span // 2):
                            sts = [ps_sc.tile([P, 2, QC], F32, tag="st",
                                              name=f"sts{h2}")
                                   for h2 in range(2)]
                            pts = [ptp.tile([P, 2, QC], F32R, tag="pt",
                                            name=f"pts{h2}")
                                   for h2 in range(2)]
                            for h2 in range(2):
                                hsl = slice(h2 * D, (h2 + 1) * D)
                                for u in range(2):
                                    i = 2 * g + u
                                    nc.tensor.matmul(
                                        sts[h2][:, u, :],
                                        lhsT=_r(kt[hsl, p, i * P:(i + 1) * P]),
                                        rhs=_r(qt[hsl, p, jsl]),
                                        start=True, stop=True)
                                nc.scalar.activation(
                                    pts[h2], sts[h2], EXP, scale=SCALE)
                                for u in range(2):
                                    di = 2 * g + u - 4 * j
                                    if di >= 0:
                                        nc.vector.tensor_mul(
                                            pts[h2][:, u, :],
                                            pts[h2][:, u, :],
                                            masks[:, di, :])
                                for u in range(2):
                                    i = 2 * g + u
                                    nc.tensor.matmul(
                                        oth[h2],
                                        lhsT=_r(v4[:, i, 2 * p + h2, :]),
                                        rhs=_r(pts[h2][:, u, :]),
                                        start=(i == 0), stop=(i == span - 1))
                        # softmax normalization
                        for h2 in range(2):
                            nw = nrm.tile([P, QC], F32R, tag="nw")
                            with nc.allow_low_precision(
                                    reason="fp32r divisor for PE broadcast"):
                                nc.vector.reciprocal(
                                    nw[D:D + 1, :], oth[h2][D:D + 1, :])
                            bc = ps_ms.tile([P, QC], F32, tag="ms")
                            nc.tensor.matmul(
                                bc[0:D, :],
                                lhsT=_r(ones_sb[D:D + 1, :]),
                                rhs=_r(nw[D:D + 1, :]),
                                start=True, stop=True)
                            bcs = nrm.tile([D, QC], F32, tag="bcs")
                            nc.vector.tensor_copy(out=bcs, in_=bc[0:D, :])
                            dst_chunk = (2 * p + h2) // 2
                            if h2 == 0:
                                nc.vector.tensor_mul(
                                    ot_all[0:D, dst_chunk, jsl],
                                    oth[h2][0:D, :], bcs)
                            else:
                                tmp = oddp.tile([D, QC], F32R, tag="odd")
                                nc.vector.tensor_mul(
                                    tmp, oth[h2][0:D, :], bcs)
                                nc.sync.dma_start(
                                    out=ot_all[D:P, dst_chunk, jsl], in_=tmp)
                    # out projection for this t-chunk
                    for cot in range(CO):
                        acc = ps_ms.tile([P, QC], F32, tag="ms")
                        for c in range(VO):
                            nc.tensor.matmul(
                                acc,
                                lhsT=_r(wproj_sb[:, c, cot * P:(cot + 1) * P]),
                                rhs=_r(ot_all[:, c, jsl]),
                                start=(c == 0), stop=(c == VO - 1))
                        og = ostg.tile([P, QC], F32, tag="og")
                        nc.vector.tensor_copy(out=og, in_=acc)
                        nc.sync.dma_start(
                            out=outT_d.ap()[cot * P:(cot + 1) * P, jsl],
                            in_=og)
    nc.compile()
    return nc


def _get_built():
    if "nc" not in _BUILT:
        _BUILT["nc"] = _build_bass()
    return _BUILT["nc"]


def _shard_inputs(x, w_attn, w_proj):
    in_maps = []
    for core in range(NCORES):
        b, g = core // 2, core % 2
        cs = slice(512 * g, 512 * (g + 1))
        in_maps.append({
            "xt": np.ascontiguousarray(np.asarray(x[b]).T),
            "wqk": np.ascontiguousarray(
                np.concatenate([w_attn[:, cs],
                                w_attn[:, C:2 * C][:, cs]], axis=1)),
            "wv": np.ascontiguousarray(w_attn[:, 2 * C:3 * C][:, cs]),
            "wproj": np.ascontiguousarray(w_proj[cs, :]),
        })
    return in_maps


def kernel(x, w_attn, w_proj, _trace=False):
    x = np.asarray(x, dtype=np.float32)
    w_attn = np.asarray(w_attn, dtype=np.float32)
    w_proj = np.asarray(w_proj, dtype=np.float32)
    nc = _get_built()
    in_maps = _shard_inputs(x, w_attn, w_proj)
    res = run_bass_kernel_spmd(
        nc, in_maps, core_ids=list(range(NCORES)), trace=_trace)
    out = np.zeros((B, T, C), np.float32)
    for core in range(NCORES):
        out[core // 2] += res.results[core]["outT"].T
    if _trace:
        kernel._last_results = res
    return out


# revision 8
# speedup vs baseline: 1.2278x; 1.2278x over previous
"""Causal self-attention (B=4, T=2048, C=1024, H=16) on 8 TRN2 NeuronCores.

Sharding: core = (batch b = core//2) x (head-group g = core%2, 8 heads each).
Megatron-style: c_attn column-parallel (each core computes Q/K/V for its 8
heads only), attention local, c_proj row-parallel (each core multiplies its
512 attention-output channels into a full (T, C) partial; host sums the two
partials per batch).

On-chip formulation (everything transposed, channels on partitions):
  qkvT = W^T x^T     : Q^T/K^T as [d, t] tiles (head pairs packed 2x64=128),
                       V as [t, d] tiles with an appended ones column.
  S^T  = K Q^T       : scores transposed [kpos, q]; K=64 contraction, the two
                       heads of a pair run concurrently in PE row-groups 0/64.
  P^T  = exp(S^T/8)  : ScalarE, causal handled by multiplicative masks on the
                       4 diagonal-band tiles per q-chunk (fully-masked tiles
                       are never computed).
  O^T  = Vaug^T P^T  : PSUM-accumulated over kpos; row 64 = softmax
                       denominator (from the ones column of Vaug).
  norm               : reciprocal of row 64, broadcast to 64 partitions via a
                       K=1 matmul, multiply; odd heads shifted to partitions
                       64..127 via SBUF->SBUF DMA.
  outT = Wp^T O^T    : row-parallel projection, written transposed.

All matmuls run as float32r (fp22 mantissa, full PE rate at moving dim 512).
"""

import numpy as np

import concourse.bass as bass
import concourse.mybir as mybir
import concourse.tile as tile
from concourse import bacc
from concourse.bass_utils import run_bass_kernel_spmd

B, T, C, H = 4, 2048, 1024, 16
D = C // H            # 64 head dim
NCORES = 8
NH = H // 2           # 8 heads per core
NP = NH // 2          # 4 head pairs per core
P = 128
QC = 512              # q/t chunk
TCH = T // QC         # 4 chunks
KTILES = T // P       # 16 kpos tiles
CO = C // P           # 8 c-chunks of the model dim
VO = (NH * D) // P    # 4 chunks of the per-core attention channels
F32 = mybir.dt.float32
F32R = mybir.dt.float32r
EXP = mybir.ActivationFunctionType.Exp
SCALE = 1.0 / 8.0     # 1/sqrt(D)

_BUILT = {}


def _r(ap):
    return ap


def _build_bass():
    nc = bacc.Bacc("TRN2", target_bir_lowering=False, debug=False)
    xt_d = nc.dram_tensor("xt", [C, T], F32, kind="ExternalInput")
    wqk_d = nc.dram_tensor("wqk", [C, 2 * NH * D], F32, kind="ExternalInput")
    wv_d = nc.dram_tensor("wv", [C, NH * D], F32, kind="ExternalInput")
    wproj_d = nc.dram_tensor("wproj", [NH * D, C], F32, kind="ExternalInput")
    outT_d = nc.dram_tensor("outT", [C, T], F32, kind="ExternalOutput")

    with tile.TileContext(nc) as tc:
        with tc.tile_pool(name="persist", bufs=1) as persist:
            qt = persist.tile([P, NP, T], F32R)              # Q^T, pair-packed
            kt = persist.tile([P, NP, T], F32R)              # K^T, pair-packed
            v4 = persist.tile([P, KTILES, NH, D + 1], F32R)  # V + ones col
            ones_sb = persist.tile([P, D], F32R)
            one = nc.const_aps.tensor
            nc.vector.tensor_copy(out=ones_sb, in_=one(1.0, (P, D)))
            nc.vector.tensor_copy(out=v4[:, :, :, D],
                                  in_=one(1.0, (P, KTILES, NH)))

            # ---------------- phase 1: QKV projections ----------------
            with tc.tile_pool(name="w1", bufs=1) as w1, \
                 tc.tile_pool(name="xtp", bufs=2) as xtp, \
                 tc.tile_pool(name="ps1", bufs=3, space="PSUM") as ps1:
                wqk_sb = w1.tile([P, CO, 2 * NH * D], F32R)
                wv_sb = w1.tile([P, CO, NH * D], F32R)
                wqk_r = wqk_d.ap().rearrange("(co ci) n -> ci co n", ci=P)
                wv_r = wv_d.ap().rearrange("(co ci) n -> ci co n", ci=P)
                for c in range(CO):
                    nc.gpsimd.dma_start(out=wqk_sb[:, c, :], in_=wqk_r[:, c, :])
                    nc.gpsimd.dma_start(out=wv_sb[:, c, :], in_=wv_r[:, c, :])

                for tch in range(TCH):
                    tsl = slice(tch * QC, (tch + 1) * QC)
                    xts = xtp.tile([P, CO, QC], F32R, tag="xts")
                    xt_r = xt_d.ap()[:, tsl].rearrange(
                        "(co ci) t -> ci co t", ci=P)
                    for c in range(CO):
                        nc.gpsimd.dma_start(
                            out=xts[:, c, :], in_=xt_r[:, c, :])
                    # Q^T / K^T col-tiles: 0..3 = Q pairs, 4..7 = K pairs
                    for ct in range(8):
                        acc = ps1.tile([P, QC], F32, tag="acc")
                        for c in range(CO):
                            nc.tensor.matmul(
                                acc,
                                lhsT=_r(wqk_sb[:, c, ct * P:(ct + 1) * P]),
                                rhs=_r(xts[:, c, :]),
                                start=(c == 0), stop=(c == CO - 1))
                        dst = qt if ct < NP else kt
                        nc.vector.tensor_copy(
                            out=dst[:, ct % NP, tsl], in_=acc)
                    # V t-tiles
                    for tt in range(QC // P):
                        acc = ps1.tile([P, NH * D], F32, tag="acc")
                        for c in range(CO):
                            nc.tensor.matmul(
                                acc,
                                lhsT=_r(xts[:, c, tt * P:(tt + 1) * P]),
                                rhs=_r(wv_sb[:, c, :]),
                                start=(c == 0), stop=(c == CO - 1))
                        nc.vector.tensor_copy(
                            out=v4[:, tch * (QC // P) + tt, :, 0:D],
                            in_=acc.rearrange("p (h d) -> p h d", h=NH))

            # ---------------- phases 2+3: attention + projection -------
            with tc.tile_pool(name="att", bufs=1) as att, \
                 tc.tile_pool(name="ptp", bufs=3) as ptp, \
                 tc.tile_pool(name="nrm", bufs=2) as nrm, \
                 tc.tile_pool(name="odd", bufs=2) as oddp, \
                 tc.tile_pool(name="ostg", bufs=3) as ostg, \
                 tc.tile_pool(name="ps_sc", bufs=2, space="PSUM") as ps_sc, \
                 tc.tile_pool(name="ps_ot", bufs=2, space="PSUM") as ps_ot, \
                 tc.tile_pool(name="ps_ms", bufs=2, space="PSUM") as ps_ms:
                ot_all = att.tile([P, VO, T], F32R)   # normalized O^T
                masks = att.tile([P, 4, QC], F32)    # diagonal-band masks
                wproj_sb = att.tile([P, VO, C], F32R)
                nc.gpsimd.dma_start(
                    out=wproj_sb,
                    in_=wproj_d.ap().rearrange("(co ci) n -> ci co n", ci=P))
                for di in range(4):
                    m = masks[:, di, :]
                    nc.vector.memset(m, 1.0)
                    # keep where qf - kp - 128*di >= 0
                    nc.gpsimd.affine_select(
                        out=m, in_=m,
                        compare_op=mybir.AluOpType.is_ge,
                        fill=0.0, base=-P * di,
                        pattern=[[1, QC]], channel_multiplier=-1)

                for j in range(TCH):          # q-chunk
                    jsl = slice(j * QC, (j + 1) * QC)
                    for p in range(NP):       # head pair
                        span = 4 * (j + 1)    # causal kpos tiles
                        oth = [ps_ot.tile([D + 1, QC], F32, tag="ot",
                                          name=f"oth{h2}")
                               for h2 in range(2)]
                        for g in range(span // 2):
                            sts = [ps_sc.tile([P, 2, QC], F32, tag="st",
                                              name=f"sts{h2}")
                                   for h2 in range(2)]
                            pts = [ptp.tile([P, 2, QC], F32R, tag="pt",
                                            name=f"pts{h2}")
                                   for h2 in range(2)]
                            for h2 in range(2):
                                hsl = slice(h2 * D, (h2 + 1) * D)
                                for u in range(2):
                                    i = 2 * g + u
                                    nc.tensor.matmul(
                                        sts[h2][:, u, :],
                                        lhsT=_r(kt[hsl, p, i * P:(i + 1) * P]),
                                        rhs=_r(qt[hsl, p, jsl]),
                                        start=True, stop=True)
                            for h2 in range(2):
                                nc.scalar.activation(
                                    pts[h2], sts[h2], EXP, scale=SCALE)
                                for u in range(2):
                                    di = 2 * g + u - 4 * j
                                    if di >= 0:
                                        nc.vector.tensor_mul(
                                            pts[h2][:, u, :],
                                            pts[h2][:, u, :],
                                            masks[:, di, :])
                            for h2 in range(2):
                                for u in range(2):
                                    i = 2 * g + u
                                    nc.tensor.matmul(
                                        oth[h2],
                                        lhsT=_r(v4[:, i, 2 * p + h2, :]),
                                        rhs=_r(pts[h2][:, u, :]),
                                        start=(i == 0), stop=(i == span - 1))
                        # softmax normalization
                        for h2 in range(2):
                            nw = nrm.tile([P, QC], F32R, tag="nw")
                            rf = nrm.tile([P, QC], F32, tag="rf")
                            # custom-DVE op mishandles 1-lane slices at
                            # base 64 -> run all 65 rows, consume row 64
                            nc.vector.reciprocal_approx_fast(
                                out=rf[0:D + 1, :], in_=oth[h2])
                            nc.vector.tensor_copy(
                                out=nw[D:D + 1, :], in_=rf[D:D + 1, :])
                            bc = ps_ms.tile([P, QC], F32, tag="ms")
                            nc.tensor.matmul(
                                bc[0:D, :],
                                lhsT=_r(ones_sb[D:D + 1, :]),
                                rhs=_r(nw[D:D + 1, :]),
                                start=True, stop=True)
                            bcs = nrm.tile([D, QC], F32, tag="bcs")
                            nc.vector.tensor_copy(out=bcs, in_=bc[0:D, :])
                            dst_chunk = (2 * p + h2) // 2
                            if h2 == 0:
                                nc.vector.tensor_mul(
                                    ot_all[0:D, dst_chunk, jsl],
                                    oth[h2][0:D, :], bcs)
                            else:
                                tmp = oddp.tile([D, QC], F32R, tag="odd")
                                nc.vector.tensor_mul(
                                    tmp, oth[h2][0:D, :], bcs)
                                nc.sync.dma_start(
                                    out=ot_all[D:P, dst_chunk, jsl], in_=tmp)
                    # out projection for this t-chunk
                    for cot in range(CO):
                        acc = ps_ms.tile([P, QC], F32, tag="ms")
                        for c in range(VO):
                            nc.tensor.matmul(
                                acc,
                                lhsT=_r(wproj_sb[:, c, cot * P:(cot + 1) * P]),
                                rhs=_r(ot_all[:, c, jsl]),
                                start=(c == 0), stop=(c == VO - 1))
                        og = ostg.tile([P, QC], F32, tag="og")
                        nc.vector.tensor_copy(out=og, in_=acc)
                        nc.sync.dma_start(
                            out=outT_d.ap()[cot * P:(cot + 1) * P, jsl],
                            in_=og)
    nc.compile()
    return nc


def _get_built():
    if "nc" not in _BUILT:
        _BUILT["nc"] = _build_bass()
    return _BUILT["nc"]


def _shard_inputs(x, w_attn, w_proj):
    in_maps = []
    for core in range(NCORES):
        b, g = core // 2, core % 2
        cs = slice(512 * g, 512 * (g + 1))
        in_maps.append({
            "xt": np.ascontiguousarray(np.asarray(x[b]).T),
            "wqk": np.ascontiguousarray(
                np.concatenate([w_attn[:, cs],
                                w_attn[:, C:2 * C][:, cs]], axis=1)),
            "wv": np.ascontiguousarray(w_attn[:, 2 * C:3 * C][:, cs]),
            "wproj": np.ascontiguousarray(w_proj[cs, :]),
        })
    return in_maps


def kernel(x, w_attn, w_proj, _trace=False):
    x = np.asarray(x, dtype=np.float32)
    w_attn = np.asarray(w_attn, dtype=np.float32)
    w_proj = np.asarray(w_proj, dtype=np.float32)
    nc = _get_built()
    in_maps = _shard_inputs(x, w_attn, w_proj)
    res = run_bass_kernel_spmd(
        nc, in_maps, core_ids=list(range(NCORES)), trace=_trace)
    out = np.zeros((B, T, C), np.float32)
    for core in range(NCORES):
        out[core // 2] += res.results[core]["outT"].T
    if _trace:
        kernel._last_results = res
    return out


# revision 10
# speedup vs baseline: 1.7947x; 1.4617x over previous
"""Causal self-attention (B=4, T=2048, C=1024, H=16) on 8 TRN2 NeuronCores.

Sharding: core = (batch b = core//2) x (head-group g = core%2, 8 heads each).
Megatron-style: c_attn column-parallel (each core computes Q/K/V for its 8
heads only), attention local, c_proj row-parallel (each core multiplies its
512 attention-output channels into a full (T, C) partial; host sums the two
partials per batch).

On-chip formulation (everything transposed, channels on partitions):
  qkvT = W^T x^T     : Q^T/K^T as [d, t] tiles (head pairs packed 2x64=128),
                       V as [t, d] tiles with an appended ones column.
  S^T  = K Q^T       : scores transposed [kpos, q]; K=64 contraction, the two
                       heads of a pair run concurrently in PE row-groups 0/64.
  P^T  = exp(S^T/8)  : ScalarE, causal handled by multiplicative masks on the
                       4 diagonal-band tiles per q-chunk (fully-masked tiles
                       are never computed).
  O^T  = Vaug^T P^T  : PSUM-accumulated over kpos; row 64 = softmax
                       denominator (from the ones column of Vaug).
  norm               : reciprocal of row 64, broadcast to 64 partitions via a
                       K=1 matmul, multiply; odd heads shifted to partitions
                       64..127 via SBUF->SBUF DMA.
  outT = Wp^T O^T    : row-parallel projection, written transposed.

All matmuls run as float32r (fp22 mantissa, full PE rate at moving dim 512).
"""

import numpy as np

import concourse.bass as bass
import concourse.mybir as mybir
import concourse.tile as tile
from concourse import bacc
from concourse.bass_utils import run_bass_kernel_spmd

B, T, C, H = 4, 2048, 1024, 16
D = C // H            # 64 head dim
NCORES = 8
NH = H // 2           # 8 heads per core
NP = NH // 2          # 4 head pairs per core
P = 128
QC = 512              # q/t chunk
TCH = T // QC         # 4 chunks
KTILES = T // P       # 16 kpos tiles
CO = C // P           # 8 c-chunks of the model dim
VO = (NH * D) // P    # 4 chunks of the per-core attention channels
F32 = mybir.dt.float32
F32R = mybir.dt.float32r
BF16 = mybir.dt.bfloat16
EXP = mybir.ActivationFunctionType.Exp
SCALE = 1.0 / 8.0     # 1/sqrt(D)

_BUILT = {}


def _r(ap):
    return ap


def _build_bass():
    nc = bacc.Bacc("TRN2", target_bir_lowering=False, debug=False)
    xt_d = nc.dram_tensor("xt", [C, T], F32, kind="ExternalInput")
    wqk_d = nc.dram_tensor("wqk", [C, 2 * NH * D], F32, kind="ExternalInput")
    wv_d = nc.dram_tensor("wv", [C, NH * D], F32, kind="ExternalInput")
    wproj_d = nc.dram_tensor("wproj", [NH * D, C], F32, kind="ExternalInput")
    outT_d = nc.dram_tensor("outT", [C, T], F32, kind="ExternalOutput")

    with tile.TileContext(nc) as tc:
        with tc.tile_pool(name="persist", bufs=1) as persist:
            qt = persist.tile([P, NP, T], BF16)              # Q^T, pair-packed
            kt = persist.tile([P, NP, T], BF16)              # K^T, pair-packed
            # V + ones col, zero-padded to 128 weight columns (FWL)
            v4 = persist.tile([P, KTILES, NH, P], BF16)
            ones_sb = persist.tile([P, D], F32R)
            one = nc.const_aps.tensor
            nc.vector.tensor_copy(out=ones_sb, in_=one(1.0, (P, D)))
            nc.vector.memset(v4, 0.0)
            nc.vector.tensor_copy(out=v4[:, :, :, D],
                                  in_=one(1.0, (P, KTILES, NH)))

            # ---------------- phase 1: QKV projections ----------------
            with tc.tile_pool(name="w1", bufs=1) as w1, \
                 tc.tile_pool(name="xtp", bufs=2) as xtp, \
                 tc.tile_pool(name="ps1", bufs=3, space="PSUM") as ps1:
                wqk_sb = w1.tile([P, CO, 2 * NH * D], F32R)
                wv_sb = w1.tile([P, CO, NH * D], F32R)
                wqk_r = wqk_d.ap().rearrange("(co ci) n -> ci co n", ci=P)
                wv_r = wv_d.ap().rearrange("(co ci) n -> ci co n", ci=P)
                for c in range(CO):
                    nc.gpsimd.dma_start(out=wqk_sb[:, c, :], in_=wqk_r[:, c, :])
                    nc.gpsimd.dma_start(out=wv_sb[:, c, :], in_=wv_r[:, c, :])

                for tch in range(TCH):
                    tsl = slice(tch * QC, (tch + 1) * QC)
                    xts = xtp.tile([P, CO, QC], F32R, tag="xts")
                    xt_r = xt_d.ap()[:, tsl].rearrange(
                        "(co ci) t -> ci co t", ci=P)
                    for c in range(CO):
                        nc.gpsimd.dma_start(
                            out=xts[:, c, :], in_=xt_r[:, c, :])
                    # Q^T / K^T col-tiles: 0..3 = Q pairs, 4..7 = K pairs
                    for ct in range(8):
                        acc = ps1.tile([P, QC], F32, tag="acc")
                        for c in range(CO):
                            nc.tensor.matmul(
                                acc,
                                lhsT=_r(wqk_sb[:, c, ct * P:(ct + 1) * P]),
                                rhs=_r(xts[:, c, :]),
                                start=(c == 0), stop=(c == CO - 1))
                        dst = qt if ct < NP else kt
                        nc.vector.tensor_copy(
                            out=dst[:, ct % NP, tsl], in_=acc)
                    # V t-tiles
                    for tt in range(QC // P):
                        acc = ps1.tile([P, NH * D], F32, tag="acc")
                        for c in range(CO):
                            nc.tensor.matmul(
                                acc,
                                lhsT=_r(xts[:, c, tt * P:(tt + 1) * P]),
                                rhs=_r(wv_sb[:, c, :]),
                                start=(c == 0), stop=(c == CO - 1))
                        nc.vector.tensor_copy(
                            out=v4[:, tch * (QC // P) + tt, :, 0:D],
                            in_=acc.rearrange("p (h d) -> p h d", h=NH))

            # ---------------- phases 2+3: attention + projection -------
            with tc.tile_pool(name="att", bufs=1) as att, \
                 tc.tile_pool(name="ptp", bufs=3) as ptp, \
                 tc.tile_pool(name="nrm", bufs=2) as nrm, \
                 tc.tile_pool(name="odd", bufs=2) as oddp, \
                 tc.tile_pool(name="ostg", bufs=3) as ostg, \
                 tc.tile_pool(name="ps_sc", bufs=2, space="PSUM") as ps_sc, \
                 tc.tile_pool(name="ps_ot", bufs=2, space="PSUM") as ps_ot, \
                 tc.tile_pool(name="ps_ms", bufs=2, space="PSUM") as ps_ms:
                ot_all = att.tile([P, VO, T], F32R)   # normalized O^T
                masks = att.tile([P, 4, QC], BF16)    # diagonal-band masks
                wproj_sb = att.tile([P, VO, C], F32R)
                nc.gpsimd.dma_start(
                    out=wproj_sb,
                    in_=wproj_d.ap().rearrange("(co ci) n -> ci co n", ci=P))
                for di in range(4):
                    m = masks[:, di, :]
                    nc.vector.memset(m, 1.0)
                    # keep where qf - kp - 128*di >= 0
                    nc.gpsimd.affine_select(
                        out=m, in_=m,
                        compare_op=mybir.AluOpType.is_ge,
                        fill=0.0, base=-P * di,
                        pattern=[[1, QC]], channel_multiplier=-1)

                for j in range(TCH):          # q-chunk
                    jsl = slice(j * QC, (j + 1) * QC)
                    for p in range(NP):       # head pair
                        span = 4 * (j + 1)    # causal kpos tiles
                        oth = [ps_ot.tile([P, QC], F32, tag="ot",
                                          name=f"oth{h2}")
                               for h2 in range(2)]
                        for g in range(span // 2):
                            sts = [ps_sc.tile([P, 2, QC], F32, tag="st",
                                              name=f"sts{h2}")
                                   for h2 in range(2)]
                            pts = [ptp.tile([P, 2, QC], BF16, tag="pt",
                                            name=f"pts{h2}")
                                   for h2 in range(2)]
                            for h2 in range(2):
                                hsl = slice(h2 * D, (h2 + 1) * D)
                                for u in range(2):
                                    i = 2 * g + u
                                    nc.tensor.matmul(
                                        sts[h2][:, u, :],
                                        lhsT=_r(kt[hsl, p, i * P:(i + 1) * P]),
                                        rhs=_r(qt[hsl, p, jsl]),
                                        start=True, stop=True)
                            for h2 in range(2):
                                nc.scalar.activation(
                                    pts[h2], sts[h2], EXP, scale=SCALE)
                                for u in range(2):
                                    di = 2 * g + u - 4 * j
                                    if di >= 0:
                                        nc.vector.tensor_mul(
                                            pts[h2][:, u, :],
                                            pts[h2][:, u, :],
                                            masks[:, di, :])
                            for h2 in range(2):
                                for u in range(2):
                                    i = 2 * g + u
                                    nc.tensor.matmul(
                                        oth[h2],
                                        lhsT=_r(v4[:, i, 2 * p + h2, :]),
                                        rhs=_r(pts[h2][:, u, :]),
                                        start=(i == 0), stop=(i == span - 1))
                        # softmax normalization
                        for h2 in range(2):
                            nw = nrm.tile([P, QC], F32R, tag="nw")
                            rf = nrm.tile([P, QC], F32, tag="rf")
                            # custom-DVE op mishandles 1-lane slices at
                            # base 64 -> run all 65 rows, consume row 64
                            nc.vector.reciprocal_approx_fast(
                                out=rf[0:D + 1, :], in_=oth[h2][0:D + 1, :])
                            nc.vector.tensor_copy(
                                out=nw[D:D + 1, :], in_=rf[D:D + 1, :])
                            bc = ps_ms.tile([P, QC], F32, tag="ms")
                            nc.tensor.matmul(
                                bc[0:D, :],
                                lhsT=_r(ones_sb[D:D + 1, :]),
                                rhs=_r(nw[D:D + 1, :]),
                                start=True, stop=True)
                            bcs = nrm.tile([D, QC], F32, tag="bcs")
                            nc.vector.tensor_copy(out=bcs, in_=bc[0:D, :])
                            dst_chunk = (2 * p + h2) // 2
                            if h2 == 0:
                                nc.vector.tensor_mul(
                                    ot_all[0:D, dst_chunk, jsl],
                                    oth[h2][0:D, :], bcs)
                            else:
                                tmp = oddp.tile([D, QC], F32R, tag="odd")
                                nc.vector.tensor_mul(
                                    tmp, oth[h2][0:D, :], bcs)
                                nc.sync.dma_start(
                                    out=ot_all[D:P, dst_chunk, jsl], in_=tmp)
                    # out projection for this t-chunk
                    for cot in range(CO):
                        acc = ps_ms.tile([P, QC], F32, tag="ms")
                        for c in range(VO):
                            nc.tensor.matmul(
                                acc,
                                lhsT=_r(wproj_sb[:, c, cot * P:(cot + 1) * P]),
                                rhs=_r(ot_all[:, c, jsl]),
                                start=(c == 0), stop=(c == VO - 1))
                        og = ostg.tile([P, QC], F32, tag="og")
                        nc.vector.tensor_copy(out=og, in_=acc)
                        nc.sync.dma_start(
                            out=outT_d.ap()[cot * P:(cot + 1) * P, jsl],
                            in_=og)
    nc.compile()
    return nc


def _get_built():
    if "nc" not in _BUILT:
        _BUILT["nc"] = _build_bass()
    return _BUILT["nc"]


def _shard_inputs(x, w_attn, w_proj):
    in_maps = []
    for core in range(NCORES):
        b, g = core // 2, core % 2
        cs = slice(512 * g, 512 * (g + 1))
        in_maps.append({
            "xt": np.ascontiguousarray(np.asarray(x[b]).T),
            "wqk": np.ascontiguousarray(
                np.concatenate([w_attn[:, cs],
                                w_attn[:, C:2 * C][:, cs]], axis=1)),
            "wv": np.ascontiguousarray(w_attn[:, 2 * C:3 * C][:, cs]),
            "wproj": np.ascontiguousarray(w_proj[cs, :]),
        })
    return in_maps


def kernel(x, w_attn, w_proj, _trace=False):
    x = np.asarray(x, dtype=np.float32)
    w_attn = np.asarray(w_attn, dtype=np.float32)
    w_proj = np.asarray(w_proj, dtype=np.float32)
    nc = _get_built()
    in_maps = _shard_inputs(x, w_attn, w_proj)
    res = run_bass_kernel_spmd(
        nc, in_maps, core_ids=list(range(NCORES)), trace=_trace)
    out = np.zeros((B, T, C), np.float32)
    for core in range(NCORES):
        out[core // 2] += res.results[core]["outT"].T
    if _trace:
        kernel._last_results = res
    return out
